# Initial kernel scaffold
#
"""MultiHeadAttention Trainium2 kernel (8-core SPMD).

Sharding: core c -> batch b = c//2, heads h0 = (c%2)*8 .. h0+8.
Each core computes, for its (batch, 8 heads):
  qT/kT = (Wq/Wk slice) @ x^T          [512, S]  (feature-major)
  v     = x @ (Wv slice)^T             [S, 512]  (natural, bf16)
  energy = q k^T + maskneg (rank-1 PE add), P = exp(energy/8) (ScalarE,
  accumulated rowsum), att = P * recip(rowsum)  -> DRAM (fp32, exact mask
  zeros via exp underflow; rowmax subtraction skipped -- energies are
  O(1) so exp cannot overflow, matches reference to fp rounding)
  attT (bf16, XBAR dma transpose) @ v -> y^T, then x_part = y^T.T @ WoT.
Host sums the two per-batch x partials + bo.
"""

import numpy as np

import concourse.bass as bass
import concourse.mybir as mybir
import concourse.tile as tile
from concourse.bass import ts
from concourse.bass_utils import run_bass_kernel_spmd

F32 = mybir.dt.float32
BF16 = mybir.dt.bfloat16

B, S_FULL, D, H = 4, 2048, 1024, 16
HD = 64
HPC = 8          # heads per core
FPC = HPC * HD   # features per core (512)
P = 128

# matmul dtype knobs: None -> plain fp32; mybir.dt.float32r -> fast fp32
QK_MM_DT = mybir.dt.float32r
PROJ_MM_DT = mybir.dt.float32r
O_MM_DT = mybir.dt.float32r


def _mm(ap, dt):
    return ap.bitcast(dt) if dt is not None else ap


def build_nc(S=S_FULL):
    nc = bass.Bass()
    ST = S // P          # q tiles
    HALF = min(1024, S)  # energy psum tile width
    NHALF = S // HALF
    NW = min(512, S)     # matmul N chunk

    xqT = nc.declare_dram_parameter("xqT", [D, S], F32)
    xkT = nc.declare_dram_parameter("xkT", [D, S], F32)
    xvT = nc.declare_dram_parameter("xvT", [D, S], F32)
    wqT = nc.declare_dram_parameter("wqT", [D, FPC], F32)
    wkT = nc.declare_dram_parameter("wkT", [D, FPC], F32)
    wvT = nc.declare_dram_parameter("wvT", [D, FPC], F32)
    woT = nc.declare_dram_parameter("woT", [FPC, D], F32)
    bq = nc.declare_dram_parameter("bq", [P, 4], F32)
    bk = nc.declare_dram_parameter("bk", [P, 4], F32)
    bvrep = nc.declare_dram_parameter("bvrep", [P, FPC], F32)
    maskneg = nc.declare_dram_parameter("maskneg", [1, S], F32)
    att_out = nc.declare_dram_parameter("att_out", [HPC, S, S], F32, isOutput=True)
    x_out = nc.declare_dram_parameter("x_out", [S, D], F32, isOutput=True)

    AF = mybir.ActivationFunctionType
    OP = mybir.AluOpType

    with tile.TileContext(nc) as tc:
        with tc.tile_pool(name="const", bufs=1) as cpool:
            qT_all = cpool.tile([P, 4, S], F32)   # [feat%128, featchunk, s]
            kT_all = cpool.tile([P, 4, S], F32)
            v_nat = cpool.tile([P, ST, FPC], BF16)  # [s%128, schunk, feat]
            mask_sb = cpool.tile([1, S], F32)
            ones_sb = cpool.tile([1, P], F32)
            bq_sb = cpool.tile([P, 4], F32)
            bk_sb = cpool.tile([P, 4], F32)
            bv_sb = cpool.tile([P, FPC], F32)

            nc.sync.dma_start(mask_sb, maskneg)
            nc.vector.memset(ones_sb, 1.0)
            nc.sync.dma_start(bq_sb, bq)
            nc.sync.dma_start(bk_sb, bk)
            nc.sync.dma_start(bv_sb, bvrep)

            # ---------------- Phase 1: projections ----------------
            with tc.tile_pool(name="wsb", bufs=1) as wpool, \
                 tc.tile_pool(name="xin", bufs=3) as xpool, \
                 tc.tile_pool(name="pproj", bufs=2, space="PSUM") as pp:
                wq_sb = wpool.tile([P, 8, FPC], F32)
                wk_sb = wpool.tile([P, 8, FPC], F32)
                wv_sb = wpool.tile([P, 8, FPC], F32)
                nc.sync.dma_start(wq_sb, wqT.rearrange("(o p) m -> p o m", p=P))
                nc.sync.dma_start(wk_sb, wkT.rearrange("(o p) m -> p o m", p=P))
                nc.sync.dma_start(wv_sb, wvT.rearrange("(o p) m -> p o m", p=P))

                for xd, wt, bias_t, dest in (
                    (xqT, wq_sb, bq_sb, qT_all),
                    (xkT, wk_sb, bk_sb, kT_all),
                ):
                    xr = xd.rearrange("(o p) s -> p o s", p=P)
                    for n in range(S // NW):
                        xt = xpool.tile([P, 8, NW], F32, tag="xin")
                        nc.sync.dma_start(xt, xr[:, :, ts(n, NW)])
                        for m in range(4):
                            ps = pp.tile([P, NW], F32, tag="pproj")
                            for kc in range(8):
                                nc.tensor.matmul(
                                    ps,
                                    lhsT=_mm(wt[:, kc, ts(m, P)], PROJ_MM_DT),
                                    rhs=_mm(xt[:, kc, :], PROJ_MM_DT),
                                    start=(kc == 0),
                                    stop=(kc == 7),
                                )
                            nc.vector.tensor_scalar_add(
                                dest[:, m, ts(n, NW)], ps, bias_t[:, m : m + 1]
                            )

                # V in natural [s, feat] layout (bf16 for AV matmul)
                xvr = xvT.rearrange("(o p) s -> p o s", p=P)
                for n in range(S // NW):
                    xt = xpool.tile([P, 8, NW], F32, tag="xin")
                    nc.sync.dma_start(xt, xvr[:, :, ts(n, NW)])
                    for sub in range(NW // P):
                        sc = n * (NW // P) + sub
                        ps = pp.tile([P, NW], F32, tag="pproj")
                        for kc in range(8):
                            nc.tensor.matmul(
                                ps[:, :FPC],
                                lhsT=_mm(xt[:, kc, ts(sub, P)], PROJ_MM_DT),
                                rhs=_mm(wv_sb[:, kc, :], PROJ_MM_DT),
                                start=(kc == 0),
                                stop=(kc == 7),
                            )
                        nc.vector.tensor_tensor(
                            v_nat[:, sc, :], ps[:, :FPC], bv_sb, OP.add
                        )

            # persistent y^T accumulator [feat%128, featchunk, s]
            with tc.tile_pool(name="ysb", bufs=1) as ypool:
                yT_all = ypool.tile([P, 4, S], F32)

                # ---------------- Phase 2: attention ----------------
                with tc.tile_pool(name="eps", bufs=2, space="PSUM") as pspool, \
                     tc.tile_pool(name="pav", bufs=2, space="PSUM") as avpool, \
                     tc.tile_pool(name="esb", bufs=2) as epool, \
                     tc.tile_pool(name="attsb", bufs=2) as apool, \
                     tc.tile_pool(name="attbf", bufs=2) as bfpool, \
                     tc.tile_pool(name="attT", bufs=2) as tpool, \
                     tc.tile_pool(name="rs", bufs=4) as rspool:
                    for h in range(HPC):
                        pb = (h % 2) * 64
                        ch = h // 2
                        for t in range(ST):
                            e_sb = epool.tile([P, S], F32, tag="esb")
                            rs = rspool.tile([P, NHALF + 2], F32, tag="rs")
                            for hf in range(NHALF):
                                ps = pspool.tile([P, HALF], F32, tag="eps")
                                for j in range(HALF // NW):
                                    k0 = hf * HALF + j * NW
                                    nc.tensor.matmul(
                                        ps[:, ts(j, NW)],
                                        lhsT=_mm(
                                            qT_all[pb : pb + HD, ch, ts(t, P)], QK_MM_DT
                                        ),
                                        rhs=_mm(
                                            kT_all[pb : pb + HD, ch, k0 : k0 + NW],
                                            QK_MM_DT,
                                        ),
                                        start=True,
                                        stop=False,
                                    )
                                    nc.tensor.matmul(
                                        ps[:, ts(j, NW)],
                                        lhsT=_mm(ones_sb, QK_MM_DT),
                                        rhs=_mm(mask_sb[:, k0 : k0 + NW], QK_MM_DT),
                                        start=False,
                                        stop=True,
                                    )
                                nc.scalar.activation(
                                    e_sb[:, ts(hf, HALF)],
                                    ps,
                                    AF.Exp,
                                    scale=0.125,
                                    accum_out=rs[:, hf : hf + 1],
                                )
                            if NHALF == 2:
                                nc.vector.tensor_tensor(
                                    rs[:, NHALF : NHALF + 1],
                                    rs[:, 0:1],
                                    rs[:, 1:2],
                                    OP.add,
                                )
                            else:
                                nc.vector.tensor_copy(
                                    out=rs[:, NHALF : NHALF + 1], in_=rs[:, 0:1]
                                )
                            nc.vector.reciprocal(
                                rs[:, NHALF + 1 : NHALF + 2],
                                rs[:, NHALF : NHALF + 1],
                            )
                            att_sb = apool.tile([P, S], F32, tag="attsb")
                            nc.vector.tensor_scalar_mul(
                                att_sb, e_sb, rs[:, NHALF + 1 : NHALF + 2]
                            )
                            nc.sync.dma_start(att_out[h, ts(t, P), :], att_sb)
                            attbf = bfpool.tile([P, S], BF16, tag="attbf")
                            nc.vector.tensor_copy(out=attbf, in_=att_sb)
                            attT = tpool.tile([P, ST, P], BF16, tag="attT")
                            nc.sync.dma_start_transpose(attT, attbf)
                            pav = avpool.tile([P, P], F32, tag="pav")
                            for kc in range(ST):
                                nc.tensor.matmul(
                                    pav[pb : pb + HD, :],
                                    lhsT=v_nat[:, kc, h * HD : (h + 1) * HD],
                                    rhs=attT[:, kc, :],
                                    start=(kc == 0),
                                    stop=(kc == ST - 1),
                                    tile_position=(0, pb),
                                )
                            nc.vector.tensor_copy(
                                out=yT_all[pb : pb + HD, ch, ts(t, P)],
                                in_=pav[pb : pb + HD, :],
                            )

                # ---------------- Phase 3: output projection ----------------
                with tc.tile_pool(name="wo", bufs=1) as wopool, \
                     tc.tile_pool(name="po", bufs=2, space="PSUM") as popool, \
                     tc.tile_pool(name="xstage", bufs=2) as xspool:
                    wo_sb = wopool.tile([P, 4, D], F32)
                    nc.sync.dma_start(wo_sb, woT.rearrange("(o p) m -> p o m", p=P))
                    for t in range(ST):
                        x_sb = xspool.tile([P, D], F32, tag="xsb")
                        for nn in range(D // 512):
                            ps = popool.tile([P, 512], F32, tag="po")
                            for kc in range(4):
                                nc.tensor.matmul(
                                    ps,
                                    lhsT=_mm(yT_all[:, kc, ts(t, P)], O_MM_DT),
                                    rhs=_mm(wo_sb[:, kc, ts(nn, 512)], O_MM_DT),
                                    start=(kc == 0),
                                    stop=(kc == 3),
                                )
                            nc.vector.tensor_copy(out=x_sb[:, ts(nn, 512)], in_=ps)
                        nc.sync.dma_start(x_out[ts(t, P), :], x_sb)

    return nc


def make_in_map(query, key, value, mask, Wq, bq, Wk, bk, Wv, bv, Wo, bo, core, S=S_FULL):
    b, hh = divmod(core, 2)
    h0 = hh * HPC
    fsl = slice(h0 * HD, h0 * HD + FPC)
    f32 = np.float32
    c = np.ascontiguousarray
    return {
        "xqT": c(query[b].T.astype(f32)),
        "xkT": c(key[b].T.astype(f32)),
        "xvT": c(value[b].T.astype(f32)),
        "wqT": c(Wq[fsl, :].T.astype(f32)),
        "wkT": c(Wk[fsl, :].T.astype(f32)),
        "wvT": c(Wv[fsl, :].T.astype(f32)),
        "woT": c(Wo[:, fsl].T.astype(f32)),
        "bq": c(np.asarray(bq)[fsl].reshape(4, P).T.astype(f32)),
        "bk": c(np.asarray(bk)[fsl].reshape(4, P).T.astype(f32)),
        "bvrep": c(np.tile(np.asarray(bv)[fsl][None, :].astype(f32), (P, 1))),
        "maskneg": c(
            ((np.asarray(mask[b, 0, 0]).astype(f32) - 1.0) * np.float32(1e10))[None, :]
        ),
    }


_NC_CACHE = {}


def kernel(query, key, value, mask, Wq, bq, Wk, bk, Wv, bv, Wo, bo):
    args = [np.asarray(a) for a in (query, key, value, mask, Wq, bq, Wk, bk, Wv, bv, Wo, bo)]
    query, key, value, mask, Wq, bq, Wk, bk, Wv, bv, Wo, bo = args
    if "nc" not in _NC_CACHE:
        _NC_CACHE["nc"] = build_nc()
    nc = _NC_CACHE["nc"]
    in_maps = [
        make_in_map(query, key, value, mask, Wq, bq, Wk, bk, Wv, bv, Wo, bo, c)
        for c in range(8)
    ]
    res = run_bass_kernel_spmd(nc, in_maps, list(range(8))).results
    attention = np.empty((B, H, S_FULL, S_FULL), np.float32)
    x = np.empty((B, S_FULL, D), np.float32)
    for c in range(8):
        b, hh = divmod(c, 2)
        attention[b, hh * HPC : (hh + 1) * HPC] = res[c]["att_out"]
    for b in range(B):
        x[b] = res[2 * b]["x_out"] + res[2 * b + 1]["x_out"] + bo[None, :].astype(
            np.float32
        )
    return x, attention


# revision 7
# speedup vs baseline: 4.3885x; 4.3885x over previous
"""MultiHeadAttention Trainium2 kernel (8-core SPMD).

Sharding: core c -> batch b = c//2, heads h0 = (c%2)*8 .. h0+8.
Each core computes, for its (batch, 8 heads):
  qT/kT = (Wq/Wk slice) @ x^T          [512, S]  (feature-major)
  v     = x @ (Wv slice)^T             [S, 512]  (natural, bf16)
  energy = q k^T + maskneg (rank-1 PE add), P = exp(energy/8) (ScalarE,
  accumulated rowsum), att = P * recip(rowsum)  -> DRAM (fp32, exact mask
  zeros via exp underflow; rowmax subtraction skipped -- energies are
  O(1) so exp cannot overflow, matches reference to fp rounding)
  attT (bf16, XBAR dma transpose) @ v -> y^T, then x_part = y^T.T @ WoT.
Host sums the two per-batch x partials + bo.
"""

import numpy as np

import concourse.bass as bass
from concourse import bacc
import concourse.mybir as mybir
import concourse.tile as tile
from concourse.bass import ts
from concourse.bass_utils import run_bass_kernel_spmd

F32 = mybir.dt.float32
BF16 = mybir.dt.bfloat16

B, S_FULL, D, H = 4, 2048, 1024, 16
HD = 64
HPC = 8          # heads per core
FPC = HPC * HD   # features per core (512)
P = 128

# matmul dtype knobs: None -> plain fp32; mybir.dt.float32r -> fast fp32
QK_MM_DT = None
PROJ_MM_DT = None
O_MM_DT = None


def _mm(ap, dt):
    return ap.bitcast(dt) if dt is not None else ap


def build_nc(S=S_FULL, bench_internal=False):
    nc = bacc.Bacc("TRN2", target_bir_lowering=False, debug=False)
    ST = S // P          # q tiles
    HALF = min(1024, S)  # energy psum tile width
    NHALF = S // HALF
    NW = min(512, S)     # matmul N chunk

    if bench_internal:
        def din(name, shape, dt):
            return nc.dram_tensor(name, shape, dt)
        dout = din
    else:
        def din(name, shape, dt):
            return nc.declare_dram_parameter(name, shape, dt, isOutput=False)
        def dout(name, shape, dt):
            return nc.declare_dram_parameter(name, shape, dt, isOutput=True)
    xqT = din("xqT", [D, S], F32)
    xkT = din("xkT", [D, S], F32)
    xvT = din("xvT", [D, S], F32)
    wqT = din("wqT", [D, FPC], F32)
    wkT = din("wkT", [D, FPC], F32)
    wvT = din("wvT", [D, FPC], F32)
    woT = din("woT", [FPC, D], F32)
    bq = din("bq", [P, 4], F32)
    bk = din("bk", [P, 4], F32)
    bvrep = din("bvrep", [P, FPC], F32)
    maskneg = din("maskneg", [1, S], F32)
    att_out = dout("att_out", [HPC, S, S], F32)
    x_out = dout("x_out", [S, D], F32)
    if bench_internal:
        dummy_in = nc.declare_dram_parameter("dummy_in", [P, P], F32, isOutput=False)
        dummy_out = nc.declare_dram_parameter("dummy_out", [P, P], F32, isOutput=True)

    AF = mybir.ActivationFunctionType
    OP = mybir.AluOpType

    with tile.TileContext(nc) as tc:
        with tc.tile_pool(name="const", bufs=1) as cpool:
            qT_all = cpool.tile([P, 4, S], F32)   # [feat%128, featchunk, s]
            kT_all = cpool.tile([P, 4, S], F32)
            v_nat = cpool.tile([P, ST, FPC], BF16)  # [s%128, schunk, feat]
            mask_sb = cpool.tile([1, S], F32)
            ones_sb = cpool.tile([1, P], F32)
            bq_sb = cpool.tile([P, 4], F32)
            bk_sb = cpool.tile([P, 4], F32)
            bv_sb = cpool.tile([P, FPC], F32)

            if bench_internal:
                dtile = cpool.tile([P, P], F32)
                nc.sync.dma_start(dtile, dummy_in[:])
                nc.sync.dma_start(dummy_out[:], dtile)
            nc.sync.dma_start(mask_sb, maskneg[:])
            nc.vector.memset(ones_sb, 1.0)
            nc.sync.dma_start(bq_sb, bq[:])
            nc.sync.dma_start(bk_sb, bk[:])
            nc.sync.dma_start(bv_sb, bvrep[:])

            # ---------------- Phase 1: projections ----------------
            with tc.tile_pool(name="wsb", bufs=1) as wpool, \
                 tc.tile_pool(name="xin", bufs=3) as xpool, \
                 tc.tile_pool(name="pproj", bufs=2, space="PSUM") as pp:
                wq_sb = wpool.tile([P, 8, FPC], F32)
                wk_sb = wpool.tile([P, 8, FPC], F32)
                wv_sb = wpool.tile([P, 8, FPC], F32)
                nc.sync.dma_start(wq_sb, wqT[:].rearrange("(o p) m -> p o m", p=P))
                nc.sync.dma_start(wk_sb, wkT[:].rearrange("(o p) m -> p o m", p=P))
                nc.sync.dma_start(wv_sb, wvT[:].rearrange("(o p) m -> p o m", p=P))

                for xd, wt, bias_t, dest in (
                    (xqT, wq_sb, bq_sb, qT_all),
                    (xkT, wk_sb, bk_sb, kT_all),
                ):
                    xr = xd[:].rearrange("(o p) s -> p o s", p=P)
                    for n in range(S // NW):
                        xt = xpool.tile([P, 8, NW], F32, tag="xin")
                        nc.sync.dma_start(xt, xr[:, :, ts(n, NW)])
                        for m in range(4):
                            ps = pp.tile([P, NW], F32, tag="pproj")
                            for kc in range(8):
                                nc.tensor.matmul(
                                    ps,
                                    lhsT=_mm(wt[:, kc, ts(m, P)], PROJ_MM_DT),
                                    rhs=_mm(xt[:, kc, :], PROJ_MM_DT),
                                    start=(kc == 0),
                                    stop=(kc == 7),
                                )
                            nc.vector.tensor_scalar_add(
                                dest[:, m, ts(n, NW)], ps, bias_t[:, m : m + 1]
                            )

                # V in natural [s, feat] layout (bf16 for AV matmul)
                xvr = xvT[:].rearrange("(o p) s -> p o s", p=P)
                for n in range(S // NW):
                    xt = xpool.tile([P, 8, NW], F32, tag="xin")
                    nc.sync.dma_start(xt, xvr[:, :, ts(n, NW)])
                    for sub in range(NW // P):
                        sc = n * (NW // P) + sub
                        ps = pp.tile([P, NW], F32, tag="pproj")
                        for kc in range(8):
                            nc.tensor.matmul(
                                ps[:, :FPC],
                                lhsT=_mm(xt[:, kc, ts(sub, P)], PROJ_MM_DT),
                                rhs=_mm(wv_sb[:, kc, :], PROJ_MM_DT),
                                start=(kc == 0),
                                stop=(kc == 7),
                            )
                        nc.vector.tensor_tensor(
                            v_nat[:, sc, :], ps[:, :FPC], bv_sb, OP.add
                        )

            # persistent y^T accumulator [feat%128, featchunk, s]
            with tc.tile_pool(name="ysb", bufs=1) as ypool:
                yT_all = ypool.tile([P, 4, S], F32)

                # ---------------- Phase 2: attention ----------------
                with tc.tile_pool(name="eps", bufs=2, space="PSUM") as pspool, \
                     tc.tile_pool(name="pav", bufs=2, space="PSUM") as avpool, \
                     tc.tile_pool(name="esb", bufs=2) as epool, \
                     tc.tile_pool(name="attsb", bufs=2) as apool, \
                     tc.tile_pool(name="attbf", bufs=2) as bfpool, \
                     tc.tile_pool(name="attT", bufs=2) as tpool, \
                     tc.tile_pool(name="rs", bufs=4) as rspool:
                    for h in range(HPC):
                        pb = (h % 2) * 64
                        ch = h // 2
                        for t in range(ST):
                            e_sb = epool.tile([P, S], F32, tag="esb")
                            rs = rspool.tile([P, NHALF + 2], F32, tag="rs")
                            for hf in range(NHALF):
                                ps = pspool.tile([P, HALF], F32, tag="eps")
                                for j in range(HALF // NW):
                                    k0 = hf * HALF + j * NW
                                    nc.tensor.matmul(
                                        ps[:, ts(j, NW)],
                                        lhsT=_mm(
                                            qT_all[pb : pb + HD, ch, ts(t, P)], QK_MM_DT
                                        ),
                                        rhs=_mm(
                                            kT_all[pb : pb + HD, ch, k0 : k0 + NW],
                                            QK_MM_DT,
                                        ),
                                        start=True,
                                        stop=False,
                                    )
                                    nc.tensor.matmul(
                                        ps[:, ts(j, NW)],
                                        lhsT=_mm(ones_sb, QK_MM_DT),
                                        rhs=_mm(mask_sb[:, k0 : k0 + NW], QK_MM_DT),
                                        start=False,
                                        stop=True,
                                    )
                                nc.scalar.activation(
                                    e_sb[:, ts(hf, HALF)],
                                    ps,
                                    AF.Exp,
                                    scale=0.125,
                                    accum_out=rs[:, hf : hf + 1],
                                )
                            if NHALF == 2:
                                nc.vector.tensor_tensor(
                                    rs[:, NHALF : NHALF + 1],
                                    rs[:, 0:1],
                                    rs[:, 1:2],
                                    OP.add,
                                )
                            else:
                                nc.vector.tensor_copy(
                                    out=rs[:, NHALF : NHALF + 1], in_=rs[:, 0:1]
                                )
                            nc.vector.reciprocal(
                                rs[:, NHALF + 1 : NHALF + 2],
                                rs[:, NHALF : NHALF + 1],
                            )
                            att_sb = apool.tile([P, S], F32, tag="attsb")
                            nc.vector.tensor_scalar_mul(
                                att_sb, e_sb, rs[:, NHALF + 1 : NHALF + 2]
                            )
                            nc.sync.dma_start(att_out[h, ts(t, P), :], att_sb)
                            attbf = bfpool.tile([P, S], BF16, tag="attbf")
                            nc.vector.tensor_copy(out=attbf, in_=att_sb)
                            attT = tpool.tile([P, ST, P], BF16, tag="attT")
                            nc.sync.dma_start_transpose(attT, attbf)
                            pav = avpool.tile([P, P], F32, tag="pav")
                            for kc in range(ST):
                                nc.tensor.matmul(
                                    pav[pb : pb + HD, :],
                                    lhsT=v_nat[:, kc, h * HD : (h + 1) * HD],
                                    rhs=attT[:, kc, :],
                                    start=(kc == 0),
                                    stop=(kc == ST - 1),
                                    tile_position=(0, pb),
                                )
                            nc.vector.tensor_copy(
                                out=yT_all[pb : pb + HD, ch, ts(t, P)],
                                in_=pav[pb : pb + HD, :],
                            )

                # ---------------- Phase 3: output projection ----------------
                with tc.tile_pool(name="wo", bufs=1) as wopool, \
                     tc.tile_pool(name="po", bufs=2, space="PSUM") as popool, \
                     tc.tile_pool(name="xstage", bufs=2) as xspool:
                    wo_sb = wopool.tile([P, 4, D], F32)
                    nc.sync.dma_start(wo_sb, woT[:].rearrange("(o p) m -> p o m", p=P))
                    for t in range(ST):
                        x_sb = xspool.tile([P, D], F32, tag="xsb")
                        for nn in range(D // 512):
                            ps = popool.tile([P, 512], F32, tag="po")
                            for kc in range(4):
                                nc.tensor.matmul(
                                    ps,
                                    lhsT=_mm(yT_all[:, kc, ts(t, P)], O_MM_DT),
                                    rhs=_mm(wo_sb[:, kc, ts(nn, 512)], O_MM_DT),
                                    start=(kc == 0),
                                    stop=(kc == 3),
                                )
                            nc.vector.tensor_copy(out=x_sb[:, ts(nn, 512)], in_=ps)
                        nc.sync.dma_start(x_out[ts(t, P), :], x_sb)

    nc.compile()
    return nc


def make_in_map(query, key, value, mask, Wq, bq, Wk, bk, Wv, bv, Wo, bo, core, S=S_FULL):
    b, hh = divmod(core, 2)
    h0 = hh * HPC
    fsl = slice(h0 * HD, h0 * HD + FPC)
    f32 = np.float32
    c = np.ascontiguousarray
    return {
        "xqT": c(query[b].T.astype(f32)),
        "xkT": c(key[b].T.astype(f32)),
        "xvT": c(value[b].T.astype(f32)),
        "wqT": c(Wq[fsl, :].T.astype(f32)),
        "wkT": c(Wk[fsl, :].T.astype(f32)),
        "wvT": c(Wv[fsl, :].T.astype(f32)),
        "woT": c(Wo[:, fsl].T.astype(f32)),
        "bq": c(np.asarray(bq)[fsl].reshape(4, P).T.astype(f32)),
        "bk": c(np.asarray(bk)[fsl].reshape(4, P).T.astype(f32)),
        "bvrep": c(np.tile(np.asarray(bv)[fsl][None, :].astype(f32), (P, 1))),
        "maskneg": c(
            ((np.asarray(mask[b, 0, 0]).astype(f32) - 1.0) * np.float32(1e10))[None, :]
        ),
    }


_NC_CACHE = {}


def kernel(query, key, value, mask, Wq, bq, Wk, bk, Wv, bv, Wo, bo):
    args = [np.asarray(a) for a in (query, key, value, mask, Wq, bq, Wk, bk, Wv, bv, Wo, bo)]
    query, key, value, mask, Wq, bq, Wk, bk, Wv, bv, Wo, bo = args
    if "nc" not in _NC_CACHE:
        _NC_CACHE["nc"] = build_nc()
    nc = _NC_CACHE["nc"]
    in_maps = [
        make_in_map(query, key, value, mask, Wq, bq, Wk, bk, Wv, bv, Wo, bo, c)
        for c in range(8)
    ]
    bkr = run_bass_kernel_spmd(nc, in_maps, list(range(8)), **_NC_CACHE.get("run_kwargs", {}))
    _NC_CACHE["last_result"] = bkr
    res = bkr.results
    attention = np.empty((B, H, S_FULL, S_FULL), np.float32)
    x = np.empty((B, S_FULL, D), np.float32)
    for c in range(8):
        b, hh = divmod(c, 2)
        attention[b, hh * HPC : (hh + 1) * HPC] = res[c]["att_out"]
    for b in range(B):
        x[b] = res[2 * b]["x_out"] + res[2 * b + 1]["x_out"] + bo[None, :].astype(
            np.float32
        )
    return x, attention
